# revision 26
# baseline (speedup 1.0000x reference)
"""Causal self-attention with RoPE on 8 Trainium2 NeuronCores.

Sharding: Megatron-style head parallelism. 16 heads / 8 cores = 2 heads per
core. Each core computes q/k/v projections for its 2 heads (column-parallel),
full causal attention for those heads, and a partial output projection
(row-parallel slice of w_o). The host sums the 8 partial outputs.

All matmul operands are fp16 (PSUM accumulation stays fp32): same PE rate as
f32r but full speed at any moving width, half the DMA bytes and half the
weight-load time. Measured rel err 8.4e-4. On-chip layout is fully
transposed (qT/kT [dh,t], scores ST[kv,q]) so P^T feeds the PV matmul
directly with no transposes. Host inputs are pre-packed so every DMA chunk
is one contiguous 64-128KB DRAM run.

Causal handling: score/exp/PV/softmax-sum operate only on the valid column
range [dg*TK, TQ) of each tile; the partially-masked 128-wide strip on the
diagonal gets one extra N=128 fp16 identity-matmul adding a -60000 triangle
(exp -> exact 0). This trims ~12% of attention-phase work vs full tiles.

Softmax denominators: full kv tiles are quad-summed on the vector engine
(3 adds per 4 tiles); each quad / diagonal tile accumulates into a [1,TQ]
PSUM row via a ones-column matmul, deferred behind newer tiles so the PE
never waits on a DVE add. 1/denom is computed as exp(-ln(d)) on the scalar
engine - Ln/Exp/Copy share one ACT table (no reloads) and this avoids the
3.4us hardware InstReciprocal; it is emitted from the deferred norm filler
so the next group's first exp stays ahead of it in the ACT queue. The
reciprocal row is broadcast to [dh,TQ] by a ones-row matmul and applied by
one DVE multiply.

Scheduling is the main lever: the PE p-state drops to ~1.2-2GHz after any
queue gap and only sustains ~2.37GHz when the in-order PE queue is dense.
Deferred PE work (denominator broadcast + output-projection matmul units)
sits in a FIFO of fillers; one is popped after every attention tile, and
PV matmuls trail their exp by two tiles. Attention-phase copies are spread
1:3 over ACT:DVE; RoPE runs as two partition-swapped PSUM-read muls plus an
aligned mul/add on DVE (walrus only allows mismatched base partitions when
one operand is PSUM).
"""

import collections

import numpy as np

B, T, D = 4, 2048, 2048
H, DH = 16, 128
NCORES = 8
HPC = H // NCORES  # heads per core
THETA = 10000.0

TT = 512   # projection t-tile (moving dim of q/k projection matmuls)
TQ = 512   # attention q-group width
TK = 128   # kv tile (contraction chunk of PV / partition dim of ST)
MASKV = -60000.0  # additive causal mask (fp16-representable; exp -> 0)


def _rope_tables(seq_len, d_head, theta):
    # Matches reference.rope_cos_sin numerics, then transposes to [dh, t]
    # and folds the rotate-half sign into sin.
    inv_freq = 1.0 / (theta ** (np.arange(0, d_head, 2, dtype=np.float32) / d_head))
    t = np.arange(seq_len, dtype=np.float32)
    freqs = np.einsum("i,j->ij", t, inv_freq)
    emb = np.concatenate([freqs, freqs], axis=-1)  # [T, dh]
    cosT = np.ascontiguousarray(np.cos(emb).astype(np.float32).T)  # [dh, T]
    sinT = np.ascontiguousarray(np.sin(emb).astype(np.float32).T)
    sgn = np.ones((d_head, 1), np.float32)
    sgn[: d_head // 2] = -1.0
    return cosT.astype(np.float16), (sinT * sgn).astype(np.float16)


def _legalize_waits(nc, mybir):
    """Walrus on this toolchain refuses more than one embedded sync wait
    per engine instruction. Hoist extra waits into standalone
    EventSemaphore instructions on the same engine queue (the sequencer
    executes them in-stream before the instruction, same gating)."""
    n = 0
    for f in nc.m.functions:
        for bb in f.blocks:
            out = []
            for inst in bb.instructions:
                si = inst.sync_info
                if (si and si.on_wait and len(si.on_wait) > 1
                        and not isinstance(inst, mybir.InstEventSemaphore)):
                    for w in si.on_wait[:-1]:
                        out.append(mybir.InstEventSemaphore(
                            name=f"WH-{n}", engine=inst.engine,
                            sync_info=mybir.SyncInfo(
                                on_wait=[w], on_update=[])))
                        n += 1
                    inst.sync_info = mybir.SyncInfo(
                        on_wait=[si.on_wait[-1]],
                        on_update=list(si.on_update))
                out.append(inst)
            bb.instructions = out
    return n


def _build_nc(b_sz, t_sz, d_sz, legalize=True):
    import concourse.bass as bass
    import concourse.tile as tile
    from concourse import mybir

    f32 = mybir.dt.float32
    f32r = mybir.dt.float32r
    f16 = mybir.dt.float16
    EXP = mybir.ActivationFunctionType.Exp
    LN = mybir.ActivationFunctionType.Ln
    BYP = mybir.AluOpType.bypass
    ADD = mybir.AluOpType.add
    MUL = mybir.AluOpType.mult

    DC = d_sz // 128         # contraction chunks
    NTT = t_sz // TT         # projection tiles
    NQG = t_sz // TQ         # q groups per (batch, head)
    NKT = t_sz // TK         # kv tiles
    KPG = TQ // TK           # kv tiles per q group (diagonal span)
    NCH = d_sz // 512        # out-projection column chunks

    nc = bass.Bass("TRN2", target_bir_lowering=False, debug=False,
                   enable_asserts=False, dynamic_dma_scratch_size=2048)

    # inputs are host-packed so every DMA chunk below is one contiguous
    # DRAM run (128KB/64KB) instead of 512B-1KB strided rows
    xT = nc.dram_tensor("xT", [b_sz, t_sz // TT, DC, 128, TT], f16,
                        kind="ExternalInput")
    wq = nc.dram_tensor("wq", [DC, 128, HPC * DH], f16, kind="ExternalInput")
    wk = nc.dram_tensor("wk", [DC, 128, HPC * DH], f16, kind="ExternalInput")
    wv = nc.dram_tensor("wv", [DC, 128, HPC * DH], f16, kind="ExternalInput")
    wo = nc.dram_tensor("wo", [HPC, NCH, 128, 512], f16,
                        kind="ExternalInput")
    cos = nc.dram_tensor("cos", [t_sz // TT, DH, TT], f16,
                         kind="ExternalInput")
    sin = nc.dram_tensor("sin", [t_sz // TT, DH, TT], f16,
                         kind="ExternalInput")
    tri = nc.dram_tensor("tri", [TK, TK], f16, kind="ExternalInput")
    idn = nc.dram_tensor("idn", [128, 128], f16, kind="ExternalInput")
    onc = nc.dram_tensor("onc", [128, 1], f16, kind="ExternalInput")
    onr = nc.dram_tensor("onr", [1, 128], f16, kind="ExternalInput")
    y = nc.dram_tensor("y", [b_sz, t_sz, d_sz], f16, kind="ExternalOutput")

    xT_r = xT.ap()
    wq_r = wq.ap()
    wk_r = wk.ap()
    wv_r = wv.ap()
    wo_r = wo.ap()
    y_r = y.ap()

    with tile.TileContext(nc) as tc:
        with (
            tc.tile_pool(name="consts", bufs=1) as consts,
            tc.tile_pool(name="wpool", bufs=1) as wpool,
            tc.tile_pool(name="qkv", bufs=1) as qkv,
            tc.tile_pool(name="xpool", bufs=3) as xpool,
            tc.tile_pool(name="rope", bufs=2) as rope,
            tc.tile_pool(name="pex", bufs=4) as pexp,
            tc.tile_pool(name="accp", bufs=3) as accp,
            tc.tile_pool(name="sax", bufs=4) as sax,
            tc.tile_pool(name="otn", bufs=16) as otnp,
            tc.tile_pool(name="ysbp", bufs=4) as ysbp,
            tc.tile_pool(name="psS", bufs=2, space="PSUM") as psS,
            tc.tile_pool(name="psO", bufs=1, space="PSUM") as psO,
            tc.tile_pool(name="psR", bufs=2, space="PSUM") as psR,
            tc.tile_pool(name="psY", bufs=3, space="PSUM") as psY,
        ):
            cos_sb = consts.tile([DH, t_sz], f16)
            sin_sb = consts.tile([DH, t_sz], f16)
            tri_sb = consts.tile([TK, TK], f16)
            idn_sb = consts.tile([128, 128], f16)
            onc_sb = consts.tile([128, 1], f16)
            onr_sb = consts.tile([1, 128], f16)

            wq_sb = wpool.tile([128, DC, HPC * DH], f16)
            wk_sb = wpool.tile([128, DC, HPC * DH], f16)
            wv_sb = wpool.tile([128, DC, HPC * DH], f16)
            wo_sb = wpool.tile([128, HPC, d_sz], f16)

            # first-needed data first: the first x tile and q/k/v weight
            # chunks feed the very first matmuls, so their DMAs go at the
            # head of every queue
            xt_first = xpool.tile([128, DC, TT], f16, tag="xt",
                                  name="xt_first")
            for dc in range(DC):
                nc.sync.dma_start(xt_first[:, dc, :], xT_r[0, 0, dc])
                nc.sync.dma_start(wq_sb[:, dc, :], wq_r[dc])
                nc.sync.dma_start(wk_sb[:, dc, :], wk_r[dc])
            for dc in range(DC):
                # v-projection starts ~14us in; keep wv out of the q/k
                # projections' DMA window
                nc.sync.dma_start(wv_sb[:, dc, :], wv_r[dc])

            def load_consts():
                # emitted after the first x tile's DMAs: nothing here is
                # needed before RoPE / attention of the first tile
                for i in range(NTT):
                    sl = slice(i * TT, (i + 1) * TT)
                    nc.sync.dma_start(cos_sb[:, sl], cos.ap()[i])
                    nc.sync.dma_start(sin_sb[:, sl], sin.ap()[i])
                nc.sync.dma_start(tri_sb[:], tri.ap())
                nc.sync.dma_start(idn_sb[:], idn.ap())
                nc.sync.dma_start(onc_sb[:], onc.ap())
                nc.sync.dma_start(onr_sb[:], onr.ap())

            def load_wo():
                # not needed until the first out-projection fillers, so
                # keep these 2MB out of the first x-tile's DMA window
                for hh in range(HPC):
                    for nch in range(NCH):
                        nsl = slice(nch * 512, (nch + 1) * 512)
                        nc.sync.dma_start(wo_sb[:, hh, nsl], wo_r[hh, nch])

            # deferred PE work units; popped between attention tiles and
            # projection groups to keep the in-order PE queue saturated
            fillers = collections.deque()

            def pop_filler():
                if fillers:
                    fillers.popleft()()

            def make_yunit(b, qi, tc2, nch, otn_pair, cp_eng):
                def yunit():
                    yp = psY.tile([TK, 512], f32, tag="y", name="yp")
                    for hh in range(HPC):
                        nc.tensor.matmul(
                            yp[:],
                            otn_pair[hh][:, tc2 * TK:(tc2 + 1) * TK],
                            wo_sb[:, hh, nch * 512:(nch + 1) * 512],
                            start=(hh == 0), stop=(hh == HPC - 1),
                        )
                    ysb = ysbp.tile([TK, 512], f16, tag="ysb", name="ysb")
                    if cp_eng == 0:
                        nc.scalar.copy(ysb[:], yp[:])
                    else:
                        nc.vector.tensor_copy(ysb[:], yp[:])
                    tq0 = qi * TQ + tc2 * TK
                    nc.sync.dma_start(
                        y_r[b, tq0:tq0 + TK, nch * 512:(nch + 1) * 512],
                        ysb[:])
                return yunit

            xt_next = None  # prefetched first x tile of the next batch
            otn_tiles = {}

            for b in range(b_sz):
                # ---------------- phase A: projections + RoPE ----------
                qT = [qkv.tile([DH, t_sz], f16, tag=f"qT{h}", name=f"qT{h}")
                      for h in range(HPC)]
                kT = [qkv.tile([DH, t_sz], f16, tag=f"kT{h}", name=f"kT{h}")
                      for h in range(HPC)]
                vv = qkv.tile([128, NKT, HPC * DH], f16, tag="vv", name="vv")

                for tt in range(NTT):
                    tsl = slice(tt * TT, (tt + 1) * TT)
                    if tt == 0:
                        if b == 0:
                            xt = xt_first
                        else:
                            xt = xt_next
                            xt_next = None
                    else:
                        xt = xt_pf
                    if tt + 1 < NTT:
                        # prefetch the next x tile now so its DMA overlaps
                        # this tile's ~21us of projection matmuls
                        xt_pf = xpool.tile([128, DC, TT], f16, tag="xt",
                                           name="xt_pf2")
                        for dc in range(DC):
                            nc.sync.dma_start(xt_pf[:, dc, :],
                                              xT_r[b, tt + 1, dc])
                    if b == 0 and tt == 0:
                        # consts (cos/sin aren't read until ~25us in) go
                        # behind the tt1 x-tile prefetch: during the
                        # upload-contended warmup window every queued MB
                        # ahead of an x tile costs PE stall time
                        load_consts()

                    if b == 0 and tt == 1:
                        # after xt(tt=1)'s own DMAs: wo is not needed until
                        # the first out-projection fillers in phase B
                        load_wo()

                    for h in range(HPC):
                        hs = slice(h * DH, (h + 1) * DH)
                        for dst, w_sb in ((qT[h], wq_sb), (kT[h], wk_sb)):
                            pp = psS.tile([TK, TQ], f32, tag="st", name="pp")
                            for dc in range(DC):
                                nc.tensor.matmul(
                                    pp[0:DH, :],
                                    w_sb[:, dc, hs],
                                    xt[:, dc, :],
                                    start=(dc == 0), stop=(dc == DC - 1),
                                )
                            # RoPE: dst = pp*cos + swap(pp)*sin_signed.
                            # The rotate-half swap needs mismatched base
                            # partitions, which walrus only allows when one
                            # operand is PSUM — so all three muls read pp
                            # from PSUM on DVE; the final all-SBUF fp16 add
                            # runs on the otherwise-idle GpSimd engine.
                            sh = rope.tile([DH, TT], f16, tag="sh", name="sh")
                            nc.vector.tensor_mul(
                                sh[0:64, :], pp[64:128, :],
                                sin_sb[0:64, tsl])
                            nc.vector.tensor_mul(
                                sh[64:128, :], pp[0:64, :],
                                sin_sb[64:128, tsl])
                            t1 = rope.tile([DH, TT], f16, tag="t1", name="t1")
                            nc.vector.scalar_tensor_tensor(
                                t1[:], pp[0:DH, :], 1.0, cos_sb[:, tsl],
                                BYP, MUL)
                            nc.vector.tensor_add(dst[:, tsl], t1[:], sh[:])
                            if len(fillers) > 12:
                                pop_filler()

                    for ts2 in range(TT // TK):
                        vp = psS.tile([TK, TQ], f32, tag="st", name="vp")
                        for dc in range(DC):
                            nc.tensor.matmul(
                                vp[:, 0:HPC * DH],
                                xt[:, dc, ts2 * TK:(ts2 + 1) * TK],
                                wv_sb[:, dc, :],
                                start=(dc == 0), stop=(dc == DC - 1),
                            )
                        kv_i = tt * (TT // TK) + ts2
                        nc.scalar.copy(vv[:, kv_i, :], vp[:, 0:HPC * DH])
                        if len(fillers) > 12:
                            pop_filler()

                # prefetch the first x tile of the next batch; by phase B
                # the input DMA queues are otherwise idle
                if b + 1 < b_sz:
                    xt_next = xpool.tile([128, DC, TT], f16, tag="xt",
                                         name="xt_pf")
                    for dc in range(DC):
                        nc.sync.dma_start(xt_next[:, dc, :],
                                          xT_r[b + 1, 0, dc])

                # ---------------- phase B: attention ------------------
                cp_rr = 0
                for h in range(HPC):
                    hs = slice(h * DH, (h + 1) * DH)
                    for qi in range(NQG):
                        nkv = KPG * (qi + 1)
                        q0 = qi * TQ
                        outp = psO.tile([DH, TQ], f32, tag="outT",
                                        name="outp")
                        sump = psR.tile([1, TQ], f32, tag="sums",
                                        name="sump")
                        sum_started = [False]
                        pend = [None]   # full-width pex awaiting its pair
                        pend2 = [None]  # pair buffer awaiting its quad
                        sum_q = []     # deferred sump matmuls

                        def sum_mm(src_ap, slo, last=False):
                            nc.tensor.matmul(
                                sump[0:1, slo:TQ], onc_sb[:],
                                src_ap[:, slo:TQ],
                                start=(not sum_started[0]), stop=last)
                            sum_started[0] = True
                        prevs = []
                        for ki in range(nkv):
                            dg = ki - KPG * qi
                            lo = max(dg, 0) * TK
                            stp = psS.tile([TK, TQ], f32, tag="st",
                                           name="stp")
                            nc.tensor.matmul(
                                stp[:, lo:TQ],
                                kT[h][:, ki * TK:(ki + 1) * TK],
                                qT[h][:, q0 + lo:q0 + TQ],
                                start=True, stop=(dg < 0),
                            )
                            if dg >= 0:
                                # additive triangle on the 128-wide strip
                                nc.tensor.matmul(
                                    stp[:, lo:lo + TK],
                                    idn_sb[:],
                                    tri_sb[:],
                                    start=False, stop=True,
                                )
                            pex = pexp.tile([TK, TQ], f16, tag="pex",
                                            name="pex")
                            nc.scalar.activation(pex[:, lo:TQ],
                                                 stp[:, lo:TQ], EXP)
                            # softmax denominators: full tiles are
                            # pair-summed on DVE (halves the add count),
                            # each pair / diagonal tile then accumulates
                            # into sump via a ones-column matmul. The
                            # matmul is deferred one tile so the PE never
                            # waits on the DVE pair-add (p-state guard).
                            if dg >= 0:
                                sum_q.append((pex, lo))
                            elif pend[0] is None:
                                pend[0] = pex
                            else:
                                pairb = accp.tile([TK, TQ], f16, tag="pair",
                                                  name="pairb")
                                nc.vector.scalar_tensor_tensor(
                                    pairb[:], pend[0][:], 1.0, pex[:],
                                    BYP, ADD)
                                pend[0] = None
                                if pend2[0] is None:
                                    pend2[0] = pairb
                                else:
                                    # quad: fold the two pair buffers so a
                                    # single ones-matmul covers 4 kv tiles
                                    nc.vector.scalar_tensor_tensor(
                                        pairb[:], pend2[0][:], 1.0,
                                        pairb[:], BYP, ADD)
                                    pend2[0] = None
                                    sum_q.append((pairb, 0))
                            if len(prevs) >= 2:
                                pk, plo, ppex = prevs.pop(0)
                                nc.tensor.matmul(
                                    outp[:, plo:TQ],
                                    vv[:, pk, hs],
                                    ppex[:, plo:TQ],
                                    start=(pk == 0), stop=False,
                                )
                            if len(sum_q) > 1:
                                s_ap, s_lo = sum_q.pop(0)
                                sum_mm(s_ap, s_lo)
                            pop_filler()
                            prevs.append((ki, lo, pex))
                        for di, (pk, plo, ppex) in enumerate(prevs):
                            nc.tensor.matmul(
                                outp[:, plo:TQ],
                                vv[:, pk, hs],
                                ppex[:, plo:TQ],
                                start=(pk == 0),
                                stop=(di == len(prevs) - 1),
                            )
                        while sum_q:
                            s_ap, s_lo = sum_q.pop(0)
                            sum_mm(s_ap, s_lo, last=(not sum_q))
                        oraw = sax.tile([DH, TQ], f16, tag="oraw",
                                        name="oraw")
                        nc.scalar.copy(oraw[:], outp[:])

                        def norm_filler(h=h, qi=qi, sump=sump,
                                        oraw=oraw, b=b):
                            # 1/denom as exp(-ln(denom)): same ACT table as
                            # Copy/Exp (no reloads), and deferred to this
                            # filler so the next group's exp0 is already
                            # ahead of it in the ACT queue
                            lnv = sax.tile([1, TQ], f32, tag="lnv",
                                           name="lnv")
                            nc.scalar.activation(lnv[0:1, :], sump[0:1, :],
                                                 LN)
                            rcp16 = sax.tile([1, TQ], f16, tag="rcp16",
                                             name="rcp16")
                            nc.scalar.activation(rcp16[0:1, :], lnv[0:1, :],
                                                 EXP, scale=-1.0)
                            rbc = psR.tile([DH, TQ], f32, tag="sums",
                                           name="rbc")
                            nc.tensor.matmul(rbc[:], onr_sb[:],
                                             rcp16[0:1, :],
                                             start=True, stop=True)
                            otn = otnp.tile([DH, TQ], f16, tag="otn",
                                            name="otn")
                            nc.vector.scalar_tensor_tensor(
                                otn[:], oraw[:], 1.0, rbc[:], BYP, MUL)
                            otn_tiles[(h, qi)] = otn
                            if h == HPC - 1:
                                nonlocal cp_rr
                                pair = (otn_tiles[(0, qi)], otn)
                                for tc2 in range(KPG):
                                    for nch in range(NCH):
                                        fillers.append(make_yunit(
                                            b, qi, tc2, nch, pair,
                                            (0 if cp_rr % 4 == 0 else 1)))
                                        cp_rr += 1

                        norm_filler.kind = "n"
                        # norm fillers go to the front (cheap, and they
                        # unblock the sump/rbc PSUM rotation) but in push
                        # order: h1's filler reads h0's otn
                        ni = 0
                        while (ni < len(fillers)
                               and getattr(fillers[ni], "kind", "y") == "n"):
                            ni += 1
                        fillers.insert(ni, norm_filler)
            # drain remaining deferred work
            while fillers:
                pop_filler()
    if legalize:
        _legalize_waits(nc, mybir)
    return nc


_NC_CACHE = {}
LAST_RESULT = None


def _get_nc(b_sz, t_sz, d_sz):
    key = (b_sz, t_sz, d_sz)
    if key not in _NC_CACHE:
        _NC_CACHE[key] = _build_nc(b_sz, t_sz, d_sz)
    return _NC_CACHE[key]


def kernel(x, w_q, w_k, w_v, w_o):
    from concourse.bass_utils import run_bass_kernel_spmd

    b_sz, t_sz, d_sz = x.shape
    scale = np.float32(1.0 / np.sqrt(DH))

    ntt, dc_n = t_sz // TT, d_sz // 128
    # pack to [B, NTT, DC, 128, TT]: each (tt, dc) chunk is one contiguous
    # 128KB DRAM run for the DMA engines
    xT = np.asarray(x, np.float32).astype(np.float16)
    xT = xT.transpose(0, 2, 1).reshape(b_sz, dc_n, 128, ntt, TT)
    xT = np.ascontiguousarray(xT.transpose(0, 3, 1, 2, 4))
    w_q = np.asarray(w_q, np.float32)
    w_k = np.asarray(w_k, np.float32)
    w_v = np.asarray(w_v, np.float32)
    w_o = np.asarray(w_o, np.float32)
    cosT, sinT = _rope_tables(t_sz, DH, THETA)
    cosP = np.ascontiguousarray(
        cosT.reshape(DH, ntt, TT).transpose(1, 0, 2))
    sinP = np.ascontiguousarray(
        sinT.reshape(DH, ntt, TT).transpose(1, 0, 2))

    def pack_w(w):  # [D, 256] -> [DC, 128, 256]
        return np.ascontiguousarray(
            w.astype(np.float16).reshape(dc_n, 128, HPC * DH))

    def pack_wo(w):  # [256, D] -> [HPC, NCH, 128, 512]
        w = w.astype(np.float16).reshape(HPC, 128, d_sz // 512, 512)
        return np.ascontiguousarray(w.transpose(0, 2, 1, 3))
    trim = np.zeros((TK, TK), np.float16)
    for r in range(TK):
        trim[r, :r] = MASKV
    ident = np.eye(128, dtype=np.float16)

    in_maps = []
    for c in range(NCORES):
        cs = slice(c * HPC * DH, (c + 1) * HPC * DH)
        in_maps.append({
            "xT": xT,
            "wq": pack_w(w_q[:, cs] * scale),
            "wk": pack_w(w_k[:, cs]),
            "wv": pack_w(w_v[:, cs]),
            "wo": pack_wo(w_o[cs, :]),
            "cos": cosP,
            "sin": sinP,
            "tri": trim,
            "idn": ident,
            "onc": np.ones((128, 1), np.float16),
            "onr": np.ones((1, 128), np.float16),
        })

    nc = _get_nc(b_sz, t_sz, d_sz)
    res = run_bass_kernel_spmd(nc, in_maps, core_ids=list(range(NCORES)))
    global LAST_RESULT
    LAST_RESULT = res

    out = res.results[0]["y"].astype(np.float32)
    for c in range(1, NCORES):
        out += res.results[c]["y"].astype(np.float32)
    return out


# revision 27
# speedup vs baseline: 1.0164x; 1.0164x over previous
"""Causal self-attention with RoPE on 8 Trainium2 NeuronCores.

Sharding: Megatron-style head parallelism. 16 heads / 8 cores = 2 heads per
core. Each core computes q/k/v projections for its 2 heads (column-parallel),
full causal attention for those heads, and a partial output projection
(row-parallel slice of w_o). The host sums the 8 partial outputs.

All matmul operands are fp16 (PSUM accumulation stays fp32): same PE rate as
f32r but full speed at any moving width, half the DMA bytes and half the
weight-load time. Measured rel err 8.4e-4. On-chip layout is fully
transposed (qT/kT [dh,t], scores ST[kv,q]) so P^T feeds the PV matmul
directly with no transposes. Host inputs are pre-packed so every DMA chunk
is one contiguous 64-128KB DRAM run.

Causal handling: score/exp/PV/softmax-sum operate only on the valid column
range [dg*TK, TQ) of each tile; the partially-masked 128-wide strip on the
diagonal gets one extra N=128 fp16 identity-matmul adding a -60000 triangle
(exp -> exact 0). This trims ~12% of attention-phase work vs full tiles.

Softmax denominators: full kv tiles are quad-summed on the vector engine
(3 adds per 4 tiles); each quad / diagonal tile accumulates into a [1,TQ]
PSUM row via a ones-column matmul, deferred behind newer tiles so the PE
never waits on a DVE add. 1/denom is computed as exp(-ln(d)) on the scalar
engine - Ln/Exp/Copy share one ACT table (no reloads) and this avoids the
3.4us hardware InstReciprocal; it is emitted from the deferred norm filler
so the next group's first exp stays ahead of it in the ACT queue. The
reciprocal row is broadcast to [dh,TQ] by a ones-row matmul and applied by
one DVE multiply.

Scheduling is the main lever: the PE p-state drops to ~1.2-2GHz after any
queue gap and only sustains ~2.37GHz when the in-order PE queue is dense.
Deferred PE work (denominator broadcast + output-projection matmul units)
sits in a FIFO of fillers; one is popped after every attention tile, and
PV matmuls trail their exp by two tiles. Attention-phase copies are spread
1:3 over ACT:DVE; RoPE runs as two partition-swapped PSUM-read muls plus an
aligned mul/add on DVE (walrus only allows mismatched base partitions when
one operand is PSUM).
"""

import collections

import numpy as np

B, T, D = 4, 2048, 2048
H, DH = 16, 128
NCORES = 8
HPC = H // NCORES  # heads per core
THETA = 10000.0

TT = 512   # projection t-tile (moving dim of q/k projection matmuls)
TQ = 512   # attention q-group width
TK = 128   # kv tile (contraction chunk of PV / partition dim of ST)
MASKV = -60000.0  # additive causal mask (fp16-representable; exp -> 0)


def _rope_tables(seq_len, d_head, theta):
    # Matches reference.rope_cos_sin numerics, then transposes to [dh, t]
    # and folds the rotate-half sign into sin.
    inv_freq = 1.0 / (theta ** (np.arange(0, d_head, 2, dtype=np.float32) / d_head))
    t = np.arange(seq_len, dtype=np.float32)
    freqs = np.einsum("i,j->ij", t, inv_freq)
    emb = np.concatenate([freqs, freqs], axis=-1)  # [T, dh]
    cosT = np.ascontiguousarray(np.cos(emb).astype(np.float32).T)  # [dh, T]
    sinT = np.ascontiguousarray(np.sin(emb).astype(np.float32).T)
    sgn = np.ones((d_head, 1), np.float32)
    sgn[: d_head // 2] = -1.0
    return cosT.astype(np.float16), (sinT * sgn).astype(np.float16)


def _legalize_waits(nc, mybir):
    """Walrus on this toolchain refuses more than one embedded sync wait
    per engine instruction. Hoist extra waits into standalone
    EventSemaphore instructions on the same engine queue (the sequencer
    executes them in-stream before the instruction, same gating)."""
    n = 0
    for f in nc.m.functions:
        for bb in f.blocks:
            out = []
            for inst in bb.instructions:
                si = inst.sync_info
                if (si and si.on_wait and len(si.on_wait) > 1
                        and not isinstance(inst, mybir.InstEventSemaphore)):
                    for w in si.on_wait[:-1]:
                        out.append(mybir.InstEventSemaphore(
                            name=f"WH-{n}", engine=inst.engine,
                            sync_info=mybir.SyncInfo(
                                on_wait=[w], on_update=[])))
                        n += 1
                    inst.sync_info = mybir.SyncInfo(
                        on_wait=[si.on_wait[-1]],
                        on_update=list(si.on_update))
                out.append(inst)
            bb.instructions = out
    return n


def _build_nc(b_sz, t_sz, d_sz, legalize=True):
    import concourse.bass as bass
    import concourse.tile as tile
    from concourse import mybir

    f32 = mybir.dt.float32
    f32r = mybir.dt.float32r
    f16 = mybir.dt.float16
    EXP = mybir.ActivationFunctionType.Exp
    LN = mybir.ActivationFunctionType.Ln
    BYP = mybir.AluOpType.bypass
    ADD = mybir.AluOpType.add
    MUL = mybir.AluOpType.mult

    DC = d_sz // 128         # contraction chunks
    NTT = t_sz // TT         # projection tiles
    NQG = t_sz // TQ         # q groups per (batch, head)
    NKT = t_sz // TK         # kv tiles
    KPG = TQ // TK           # kv tiles per q group (diagonal span)
    NCH = d_sz // 512        # out-projection column chunks

    nc = bass.Bass("TRN2", target_bir_lowering=False, debug=False,
                   enable_asserts=False, dynamic_dma_scratch_size=2048)

    # inputs are host-packed so every DMA chunk below is one contiguous
    # DRAM run (128KB/64KB) instead of 512B-1KB strided rows
    xT = nc.dram_tensor("xT", [b_sz, t_sz // TT, DC, 128, TT], f16,
                        kind="ExternalInput")
    wq = nc.dram_tensor("wq", [DC, 128, HPC * DH], f16, kind="ExternalInput")
    wk = nc.dram_tensor("wk", [DC, 128, HPC * DH], f16, kind="ExternalInput")
    wv = nc.dram_tensor("wv", [DC, 128, HPC * DH], f16, kind="ExternalInput")
    wo = nc.dram_tensor("wo", [HPC, NCH, 128, 512], f16,
                        kind="ExternalInput")
    cos = nc.dram_tensor("cos", [t_sz // TT, DH, TT], f16,
                         kind="ExternalInput")
    sin = nc.dram_tensor("sin", [t_sz // TT, DH, TT], f16,
                         kind="ExternalInput")
    tri = nc.dram_tensor("tri", [TK, TK], f16, kind="ExternalInput")
    idn = nc.dram_tensor("idn", [128, 128], f16, kind="ExternalInput")
    onc = nc.dram_tensor("onc", [128, 1], f16, kind="ExternalInput")
    onr = nc.dram_tensor("onr", [1, 128], f16, kind="ExternalInput")
    y = nc.dram_tensor("y", [b_sz, t_sz, d_sz], f16, kind="ExternalOutput")

    xT_r = xT.ap()
    wq_r = wq.ap()
    wk_r = wk.ap()
    wv_r = wv.ap()
    wo_r = wo.ap()
    y_r = y.ap()

    with tile.TileContext(nc) as tc:
        with (
            tc.tile_pool(name="consts", bufs=1) as consts,
            tc.tile_pool(name="wpool", bufs=1) as wpool,
            tc.tile_pool(name="qkv", bufs=1) as qkv,
            tc.tile_pool(name="xpool", bufs=3) as xpool,
            tc.tile_pool(name="rope", bufs=2) as rope,
            tc.tile_pool(name="pex", bufs=4) as pexp,
            tc.tile_pool(name="accp", bufs=3) as accp,
            tc.tile_pool(name="sax", bufs=4) as sax,
            tc.tile_pool(name="otn", bufs=16) as otnp,
            tc.tile_pool(name="ysbp", bufs=4) as ysbp,
            tc.tile_pool(name="psS", bufs=2, space="PSUM") as psS,
            tc.tile_pool(name="psO", bufs=1, space="PSUM") as psO,
            tc.tile_pool(name="psR", bufs=2, space="PSUM") as psR,
            tc.tile_pool(name="psY", bufs=3, space="PSUM") as psY,
        ):
            cos_sb = consts.tile([DH, t_sz], f16)
            sin_sb = consts.tile([DH, t_sz], f16)
            tri_sb = consts.tile([TK, TK], f16)
            idn_sb = consts.tile([128, 128], f16)
            onc_sb = consts.tile([128, 1], f16)
            onr_sb = consts.tile([1, 128], f16)

            wq_sb = wpool.tile([128, DC, HPC * DH], f16)
            wk_sb = wpool.tile([128, DC, HPC * DH], f16)
            wv_sb = wpool.tile([128, DC, HPC * DH], f16)
            wo_sb = wpool.tile([128, HPC, d_sz], f16)

            # first-needed data first: the first x tile and q/k/v weight
            # chunks feed the very first matmuls, so their DMAs go at the
            # head of every queue
            xt_first = xpool.tile([128, DC, TT], f16, tag="xt",
                                  name="xt_first")
            for dc in range(DC):
                nc.sync.dma_start(xt_first[:, dc, :], xT_r[0, 0, dc])
                nc.sync.dma_start(wq_sb[:, dc, :], wq_r[dc])
                nc.sync.dma_start(wk_sb[:, dc, :], wk_r[dc])
            for dc in range(DC):
                # v-projection starts ~14us in; keep wv out of the q/k
                # projections' DMA window
                nc.sync.dma_start(wv_sb[:, dc, :], wv_r[dc])

            def load_consts():
                # emitted after the first x tile's DMAs: nothing here is
                # needed before RoPE / attention of the first tile
                for i in range(NTT):
                    sl = slice(i * TT, (i + 1) * TT)
                    nc.sync.dma_start(cos_sb[:, sl], cos.ap()[i])
                    nc.sync.dma_start(sin_sb[:, sl], sin.ap()[i])
                nc.sync.dma_start(tri_sb[:], tri.ap())
                nc.sync.dma_start(idn_sb[:], idn.ap())
                nc.sync.dma_start(onc_sb[:], onc.ap())
                nc.sync.dma_start(onr_sb[:], onr.ap())

            def load_wo():
                # not needed until the first out-projection fillers, so
                # keep these 2MB out of the first x-tile's DMA window
                for hh in range(HPC):
                    for nch in range(NCH):
                        nsl = slice(nch * 512, (nch + 1) * 512)
                        nc.sync.dma_start(wo_sb[:, hh, nsl], wo_r[hh, nch])

            # deferred PE work units; popped between attention tiles and
            # projection groups to keep the in-order PE queue saturated
            fillers = collections.deque()

            def pop_filler():
                if fillers:
                    fillers.popleft()()

            def make_yunit(b, qi, tc2, nch, otn_pair, cp_eng):
                def yunit():
                    yp = psY.tile([TK, 512], f32, tag="y", name="yp")
                    for hh in range(HPC):
                        nc.tensor.matmul(
                            yp[:],
                            otn_pair[hh][:, tc2 * TK:(tc2 + 1) * TK],
                            wo_sb[:, hh, nch * 512:(nch + 1) * 512],
                            start=(hh == 0), stop=(hh == HPC - 1),
                        )
                    ysb = ysbp.tile([TK, 512], f16, tag="ysb", name="ysb")
                    if cp_eng == 0:
                        nc.scalar.copy(ysb[:], yp[:])
                    else:
                        nc.vector.tensor_copy(ysb[:], yp[:])
                    tq0 = qi * TQ + tc2 * TK
                    nc.sync.dma_start(
                        y_r[b, tq0:tq0 + TK, nch * 512:(nch + 1) * 512],
                        ysb[:])
                return yunit

            xt_next = None  # prefetched first x tile of the next batch
            otn_tiles = {}

            for b in range(b_sz):
                # ---------------- phase A: projections + RoPE ----------
                qT = [qkv.tile([DH, t_sz], f16, tag=f"qT{h}", name=f"qT{h}")
                      for h in range(HPC)]
                kT = [qkv.tile([DH, t_sz], f16, tag=f"kT{h}", name=f"kT{h}")
                      for h in range(HPC)]
                vv = qkv.tile([128, NKT, HPC * DH], f16, tag="vv", name="vv")

                for tt in range(NTT):
                    tsl = slice(tt * TT, (tt + 1) * TT)
                    if tt == 0:
                        if b == 0:
                            xt = xt_first
                            load_consts()
                        else:
                            xt = xt_next
                            xt_next = None
                    else:
                        xt = xt_pf
                    if tt + 1 < NTT:
                        # prefetch the next x tile now so its DMA overlaps
                        # this tile's ~21us of projection matmuls
                        xt_pf = xpool.tile([128, DC, TT], f16, tag="xt",
                                           name="xt_pf2")
                        for dc in range(DC):
                            nc.sync.dma_start(xt_pf[:, dc, :],
                                              xT_r[b, tt + 1, dc])

                    if b == 0 and tt == 1:
                        # after xt(tt=1)'s own DMAs: wo is not needed until
                        # the first out-projection fillers in phase B
                        load_wo()

                    for h in range(HPC):
                        hs = slice(h * DH, (h + 1) * DH)
                        for dst, w_sb in ((qT[h], wq_sb), (kT[h], wk_sb)):
                            pp = psS.tile([TK, TQ], f32, tag="st", name="pp")
                            for dc in range(DC):
                                nc.tensor.matmul(
                                    pp[0:DH, :],
                                    w_sb[:, dc, hs],
                                    xt[:, dc, :],
                                    start=(dc == 0), stop=(dc == DC - 1),
                                )
                            # RoPE: dst = pp*cos + swap(pp)*sin_signed.
                            # The rotate-half swap needs mismatched base
                            # partitions, which walrus only allows when one
                            # operand is PSUM — so all three muls read pp
                            # from PSUM on DVE; the final all-SBUF fp16 add
                            # runs on the otherwise-idle GpSimd engine.
                            sh = rope.tile([DH, TT], f16, tag="sh", name="sh")
                            nc.vector.tensor_mul(
                                sh[0:64, :], pp[64:128, :],
                                sin_sb[0:64, tsl])
                            nc.vector.tensor_mul(
                                sh[64:128, :], pp[0:64, :],
                                sin_sb[64:128, tsl])
                            t1 = rope.tile([DH, TT], f16, tag="t1", name="t1")
                            nc.vector.scalar_tensor_tensor(
                                t1[:], pp[0:DH, :], 1.0, cos_sb[:, tsl],
                                BYP, MUL)
                            nc.vector.tensor_add(dst[:, tsl], t1[:], sh[:])
                            if len(fillers) > 12:
                                pop_filler()

                    for ts2 in range(TT // TK):
                        vp = psS.tile([TK, TQ], f32, tag="st", name="vp")
                        for dc in range(DC):
                            nc.tensor.matmul(
                                vp[:, 0:HPC * DH],
                                xt[:, dc, ts2 * TK:(ts2 + 1) * TK],
                                wv_sb[:, dc, :],
                                start=(dc == 0), stop=(dc == DC - 1),
                            )
                        kv_i = tt * (TT // TK) + ts2
                        nc.scalar.copy(vv[:, kv_i, :], vp[:, 0:HPC * DH])
                        if len(fillers) > 12:
                            pop_filler()

                # prefetch the first x tile of the next batch; by phase B
                # the input DMA queues are otherwise idle
                if b + 1 < b_sz:
                    xt_next = xpool.tile([128, DC, TT], f16, tag="xt",
                                         name="xt_pf")
                    for dc in range(DC):
                        nc.sync.dma_start(xt_next[:, dc, :],
                                          xT_r[b + 1, 0, dc])

                # ---------------- phase B: attention ------------------
                cp_rr = 0
                for h in range(HPC):
                    hs = slice(h * DH, (h + 1) * DH)
                    for qi in range(NQG):
                        nkv = KPG * (qi + 1)
                        q0 = qi * TQ
                        outp = psO.tile([DH, TQ], f32, tag="outT",
                                        name="outp")
                        sump = psR.tile([1, TQ], f32, tag="sums",
                                        name="sump")
                        sum_started = [False]
                        pend = [None]   # full-width pex awaiting its pair
                        pend2 = [None]  # pair buffer awaiting its quad
                        sum_q = []     # deferred sump matmuls

                        def sum_mm(src_ap, slo, last=False):
                            nc.tensor.matmul(
                                sump[0:1, slo:TQ], onc_sb[:],
                                src_ap[:, slo:TQ],
                                start=(not sum_started[0]), stop=last)
                            sum_started[0] = True
                        prevs = []
                        for ki in range(nkv):
                            dg = ki - KPG * qi
                            lo = max(dg, 0) * TK
                            stp = psS.tile([TK, TQ], f32, tag="st",
                                           name="stp")
                            nc.tensor.matmul(
                                stp[:, lo:TQ],
                                kT[h][:, ki * TK:(ki + 1) * TK],
                                qT[h][:, q0 + lo:q0 + TQ],
                                start=True, stop=(dg < 0),
                            )
                            if dg >= 0:
                                # additive triangle on the 128-wide strip
                                nc.tensor.matmul(
                                    stp[:, lo:lo + TK],
                                    idn_sb[:],
                                    tri_sb[:],
                                    start=False, stop=True,
                                )
                            pex = pexp.tile([TK, TQ], f16, tag="pex",
                                            name="pex")
                            nc.scalar.activation(pex[:, lo:TQ],
                                                 stp[:, lo:TQ], EXP)
                            # softmax denominators: full tiles are
                            # pair-summed on DVE (halves the add count),
                            # each pair / diagonal tile then accumulates
                            # into sump via a ones-column matmul. The
                            # matmul is deferred one tile so the PE never
                            # waits on the DVE pair-add (p-state guard).
                            if dg >= 0:
                                sum_q.append((pex, lo))
                            elif pend[0] is None:
                                pend[0] = pex
                            else:
                                pairb = accp.tile([TK, TQ], f16, tag="pair",
                                                  name="pairb")
                                nc.vector.scalar_tensor_tensor(
                                    pairb[:], pend[0][:], 1.0, pex[:],
                                    BYP, ADD)
                                pend[0] = None
                                if pend2[0] is None:
                                    pend2[0] = pairb
                                else:
                                    # quad: fold the two pair buffers so a
                                    # single ones-matmul covers 4 kv tiles
                                    nc.vector.scalar_tensor_tensor(
                                        pairb[:], pend2[0][:], 1.0,
                                        pairb[:], BYP, ADD)
                                    pend2[0] = None
                                    sum_q.append((pairb, 0))
                            if len(prevs) >= 2:
                                pk, plo, ppex = prevs.pop(0)
                                nc.tensor.matmul(
                                    outp[:, plo:TQ],
                                    vv[:, pk, hs],
                                    ppex[:, plo:TQ],
                                    start=(pk == 0), stop=False,
                                )
                            if len(sum_q) > 1:
                                s_ap, s_lo = sum_q.pop(0)
                                sum_mm(s_ap, s_lo)
                            pop_filler()
                            prevs.append((ki, lo, pex))
                        for di, (pk, plo, ppex) in enumerate(prevs):
                            nc.tensor.matmul(
                                outp[:, plo:TQ],
                                vv[:, pk, hs],
                                ppex[:, plo:TQ],
                                start=(pk == 0),
                                stop=(di == len(prevs) - 1),
                            )
                        while sum_q:
                            s_ap, s_lo = sum_q.pop(0)
                            sum_mm(s_ap, s_lo, last=(not sum_q))
                        oraw = sax.tile([DH, TQ], f16, tag="oraw",
                                        name="oraw")
                        nc.scalar.copy(oraw[:], outp[:])

                        def norm_filler(h=h, qi=qi, sump=sump,
                                        oraw=oraw, b=b):
                            # 1/denom as exp(-ln(denom)): same ACT table as
                            # Copy/Exp (no reloads), and deferred to this
                            # filler so the next group's exp0 is already
                            # ahead of it in the ACT queue
                            lnv = sax.tile([1, TQ], f32, tag="lnv",
                                           name="lnv")
                            nc.scalar.activation(lnv[0:1, :], sump[0:1, :],
                                                 LN)
                            rcp16 = sax.tile([1, TQ], f16, tag="rcp16",
                                             name="rcp16")
                            nc.scalar.activation(rcp16[0:1, :], lnv[0:1, :],
                                                 EXP, scale=-1.0)
                            rbc = psR.tile([DH, TQ], f32, tag="sums",
                                           name="rbc")
                            nc.tensor.matmul(rbc[:], onr_sb[:],
                                             rcp16[0:1, :],
                                             start=True, stop=True)
                            otn = otnp.tile([DH, TQ], f16, tag="otn",
                                            name="otn")
                            nc.vector.scalar_tensor_tensor(
                                otn[:], oraw[:], 1.0, rbc[:], BYP, MUL)
                            otn_tiles[(h, qi)] = otn
                            if h == HPC - 1:
                                nonlocal cp_rr
                                pair = (otn_tiles[(0, qi)], otn)
                                for tc2 in range(KPG):
                                    for nch in range(NCH):
                                        fillers.append(make_yunit(
                                            b, qi, tc2, nch, pair,
                                            (0 if cp_rr % 4 == 0 else 1)))
                                        cp_rr += 1

                        norm_filler.kind = "n"
                        # norm fillers go to the front (cheap, and they
                        # unblock the sump/rbc PSUM rotation) but in push
                        # order: h1's filler reads h0's otn
                        ni = 0
                        while (ni < len(fillers)
                               and getattr(fillers[ni], "kind", "y") == "n"):
                            ni += 1
                        fillers.insert(ni, norm_filler)
            # drain remaining deferred work
            while fillers:
                pop_filler()
    if legalize:
        _legalize_waits(nc, mybir)
    return nc


_NC_CACHE = {}
LAST_RESULT = None


def _get_nc(b_sz, t_sz, d_sz):
    key = (b_sz, t_sz, d_sz)
    if key not in _NC_CACHE:
        _NC_CACHE[key] = _build_nc(b_sz, t_sz, d_sz)
    return _NC_CACHE[key]


def kernel(x, w_q, w_k, w_v, w_o):
    from concourse.bass_utils import run_bass_kernel_spmd

    b_sz, t_sz, d_sz = x.shape
    scale = np.float32(1.0 / np.sqrt(DH))

    ntt, dc_n = t_sz // TT, d_sz // 128
    # pack to [B, NTT, DC, 128, TT]: each (tt, dc) chunk is one contiguous
    # 128KB DRAM run for the DMA engines
    xT = np.asarray(x, np.float32).astype(np.float16)
    xT = xT.transpose(0, 2, 1).reshape(b_sz, dc_n, 128, ntt, TT)
    xT = np.ascontiguousarray(xT.transpose(0, 3, 1, 2, 4))
    w_q = np.asarray(w_q, np.float32)
    w_k = np.asarray(w_k, np.float32)
    w_v = np.asarray(w_v, np.float32)
    w_o = np.asarray(w_o, np.float32)
    cosT, sinT = _rope_tables(t_sz, DH, THETA)
    cosP = np.ascontiguousarray(
        cosT.reshape(DH, ntt, TT).transpose(1, 0, 2))
    sinP = np.ascontiguousarray(
        sinT.reshape(DH, ntt, TT).transpose(1, 0, 2))

    def pack_w(w):  # [D, 256] -> [DC, 128, 256]
        return np.ascontiguousarray(
            w.astype(np.float16).reshape(dc_n, 128, HPC * DH))

    def pack_wo(w):  # [256, D] -> [HPC, NCH, 128, 512]
        w = w.astype(np.float16).reshape(HPC, 128, d_sz // 512, 512)
        return np.ascontiguousarray(w.transpose(0, 2, 1, 3))
    trim = np.zeros((TK, TK), np.float16)
    for r in range(TK):
        trim[r, :r] = MASKV
    ident = np.eye(128, dtype=np.float16)

    in_maps = []
    for c in range(NCORES):
        cs = slice(c * HPC * DH, (c + 1) * HPC * DH)
        in_maps.append({
            "xT": xT,
            "wq": pack_w(w_q[:, cs] * scale),
            "wk": pack_w(w_k[:, cs]),
            "wv": pack_w(w_v[:, cs]),
            "wo": pack_wo(w_o[cs, :]),
            "cos": cosP,
            "sin": sinP,
            "tri": trim,
            "idn": ident,
            "onc": np.ones((128, 1), np.float16),
            "onr": np.ones((1, 128), np.float16),
        })

    nc = _get_nc(b_sz, t_sz, d_sz)
    res = run_bass_kernel_spmd(nc, in_maps, core_ids=list(range(NCORES)))
    global LAST_RESULT
    LAST_RESULT = res

    out = res.results[0]["y"].astype(np.float32)
    for c in range(1, NCORES):
        out += res.results[c]["y"].astype(np.float32)
    return out


# revision 28
# speedup vs baseline: 1.0204x; 1.0039x over previous
"""Causal self-attention with RoPE on 8 Trainium2 NeuronCores.

Sharding: Megatron-style head parallelism. 16 heads / 8 cores = 2 heads per
core. Each core computes q/k/v projections for its 2 heads (column-parallel),
full causal attention for those heads, and a partial output projection
(row-parallel slice of w_o). The host sums the 8 partial outputs.

All matmul operands are fp16 (PSUM accumulation stays fp32): same PE rate as
f32r but full speed at any moving width, half the DMA bytes and half the
weight-load time. Measured rel err 8.4e-4. On-chip layout is fully
transposed (qT/kT [dh,t], scores ST[kv,q]) so P^T feeds the PV matmul
directly with no transposes. Host inputs are pre-packed so every DMA chunk
is one contiguous 64-128KB DRAM run.

Causal handling: score/exp/PV/softmax-sum operate only on the valid column
range [dg*TK, TQ) of each tile; the partially-masked 128-wide strip on the
diagonal gets one extra N=128 fp16 identity-matmul adding a -60000 triangle
(exp -> exact 0). This trims ~12% of attention-phase work vs full tiles.

Softmax denominators: full kv tiles are quad-summed on the vector engine
(3 adds per 4 tiles); each quad / diagonal tile accumulates into a [1,TQ]
PSUM row via a ones-column matmul, deferred behind newer tiles so the PE
never waits on a DVE add. 1/denom is computed as exp(-ln(d)) on the scalar
engine - Ln/Exp/Copy share one ACT table (no reloads) and this avoids the
3.4us hardware InstReciprocal; it is emitted from the deferred norm filler
so the next group's first exp stays ahead of it in the ACT queue. The
reciprocal row is broadcast to [dh,TQ] by a ones-row matmul and applied by
one DVE multiply.

Scheduling is the main lever: the PE p-state drops to ~1.2-2GHz after any
queue gap and only sustains ~2.37GHz when the in-order PE queue is dense.
Deferred PE work (denominator broadcast + output-projection matmul units)
sits in a FIFO of fillers; one is popped after every attention tile, and
PV matmuls trail their exp by two tiles. Attention-phase copies are spread
1:3 over ACT:DVE; RoPE runs as two partition-swapped PSUM-read muls plus an
aligned mul/add on DVE (walrus only allows mismatched base partitions when
one operand is PSUM).
"""

import collections

import numpy as np

B, T, D = 4, 2048, 2048
H, DH = 16, 128
NCORES = 8
HPC = H // NCORES  # heads per core
THETA = 10000.0

TT = 512   # projection t-tile (moving dim of q/k projection matmuls)
TQ = 512   # attention q-group width
TK = 128   # kv tile (contraction chunk of PV / partition dim of ST)
MASKV = -60000.0  # additive causal mask (fp16-representable; exp -> 0)


def _rope_tables(seq_len, d_head, theta):
    # Matches reference.rope_cos_sin numerics, then transposes to [dh, t]
    # and folds the rotate-half sign into sin.
    inv_freq = 1.0 / (theta ** (np.arange(0, d_head, 2, dtype=np.float32) / d_head))
    t = np.arange(seq_len, dtype=np.float32)
    freqs = np.einsum("i,j->ij", t, inv_freq)
    emb = np.concatenate([freqs, freqs], axis=-1)  # [T, dh]
    cosT = np.ascontiguousarray(np.cos(emb).astype(np.float32).T)  # [dh, T]
    sinT = np.ascontiguousarray(np.sin(emb).astype(np.float32).T)
    sgn = np.ones((d_head, 1), np.float32)
    sgn[: d_head // 2] = -1.0
    return cosT.astype(np.float16), (sinT * sgn).astype(np.float16)


def _legalize_waits(nc, mybir):
    """Walrus on this toolchain refuses more than one embedded sync wait
    per engine instruction. Hoist extra waits into standalone
    EventSemaphore instructions on the same engine queue (the sequencer
    executes them in-stream before the instruction, same gating)."""
    n = 0
    for f in nc.m.functions:
        for bb in f.blocks:
            out = []
            for inst in bb.instructions:
                si = inst.sync_info
                if (si and si.on_wait and len(si.on_wait) > 1
                        and not isinstance(inst, mybir.InstEventSemaphore)):
                    for w in si.on_wait[:-1]:
                        out.append(mybir.InstEventSemaphore(
                            name=f"WH-{n}", engine=inst.engine,
                            sync_info=mybir.SyncInfo(
                                on_wait=[w], on_update=[])))
                        n += 1
                    inst.sync_info = mybir.SyncInfo(
                        on_wait=[si.on_wait[-1]],
                        on_update=list(si.on_update))
                out.append(inst)
            bb.instructions = out
    return n


def _build_nc(b_sz, t_sz, d_sz, legalize=True):
    import concourse.bass as bass
    import concourse.tile as tile
    from concourse import mybir

    f32 = mybir.dt.float32
    f32r = mybir.dt.float32r
    f16 = mybir.dt.float16
    EXP = mybir.ActivationFunctionType.Exp
    LN = mybir.ActivationFunctionType.Ln
    BYP = mybir.AluOpType.bypass
    ADD = mybir.AluOpType.add
    MUL = mybir.AluOpType.mult

    DC = d_sz // 128         # contraction chunks
    NTT = t_sz // TT         # projection tiles
    NQG = t_sz // TQ         # q groups per (batch, head)
    NKT = t_sz // TK         # kv tiles
    KPG = TQ // TK           # kv tiles per q group (diagonal span)
    NCH = d_sz // 512        # out-projection column chunks

    nc = bass.Bass("TRN2", target_bir_lowering=False, debug=False,
                   enable_asserts=False, dynamic_dma_scratch_size=2048)

    # inputs are host-packed so every DMA chunk below is one contiguous
    # DRAM run (128KB/64KB) instead of 512B-1KB strided rows
    xT = nc.dram_tensor("xT", [b_sz, t_sz // TT, DC, 128, TT], f16,
                        kind="ExternalInput")
    wq = nc.dram_tensor("wq", [DC, 128, HPC * DH], f16, kind="ExternalInput")
    wk = nc.dram_tensor("wk", [DC, 128, HPC * DH], f16, kind="ExternalInput")
    wv = nc.dram_tensor("wv", [DC, 128, HPC * DH], f16, kind="ExternalInput")
    wo = nc.dram_tensor("wo", [HPC, NCH, 128, 512], f16,
                        kind="ExternalInput")
    cos = nc.dram_tensor("cos", [t_sz // TT, DH, TT], f16,
                         kind="ExternalInput")
    sin = nc.dram_tensor("sin", [t_sz // TT, DH, TT], f16,
                         kind="ExternalInput")
    tri = nc.dram_tensor("tri", [TK, TK], f16, kind="ExternalInput")
    idn = nc.dram_tensor("idn", [128, 128], f16, kind="ExternalInput")
    onc = nc.dram_tensor("onc", [128, 1], f16, kind="ExternalInput")
    onr = nc.dram_tensor("onr", [1, 128], f16, kind="ExternalInput")
    y = nc.dram_tensor("y", [b_sz, t_sz, d_sz], f16, kind="ExternalOutput")

    xT_r = xT.ap()
    wq_r = wq.ap()
    wk_r = wk.ap()
    wv_r = wv.ap()
    wo_r = wo.ap()
    y_r = y.ap()

    with tile.TileContext(nc) as tc:
        with (
            tc.tile_pool(name="consts", bufs=1) as consts,
            tc.tile_pool(name="wpool", bufs=1) as wpool,
            tc.tile_pool(name="qkv", bufs=1) as qkv,
            tc.tile_pool(name="xpool", bufs=3) as xpool,
            tc.tile_pool(name="rope", bufs=2) as rope,
            tc.tile_pool(name="pex", bufs=5) as pexp,
            tc.tile_pool(name="accp", bufs=3) as accp,
            tc.tile_pool(name="sax", bufs=4) as sax,
            tc.tile_pool(name="otn", bufs=16) as otnp,
            tc.tile_pool(name="ysbp", bufs=4) as ysbp,
            tc.tile_pool(name="psS", bufs=2, space="PSUM") as psS,
            tc.tile_pool(name="psO", bufs=1, space="PSUM") as psO,
            tc.tile_pool(name="psR", bufs=2, space="PSUM") as psR,
            tc.tile_pool(name="psY", bufs=3, space="PSUM") as psY,
        ):
            cos_sb = consts.tile([DH, t_sz], f16)
            sin_sb = consts.tile([DH, t_sz], f16)
            tri_sb = consts.tile([TK, TK], f16)
            idn_sb = consts.tile([128, 128], f16)
            onc_sb = consts.tile([128, 1], f16)
            onr_sb = consts.tile([1, 128], f16)

            wq_sb = wpool.tile([128, DC, HPC * DH], f16)
            wk_sb = wpool.tile([128, DC, HPC * DH], f16)
            wv_sb = wpool.tile([128, DC, HPC * DH], f16)
            wo_sb = wpool.tile([128, HPC, d_sz], f16)

            # first-needed data first: the first x tile and q/k/v weight
            # chunks feed the very first matmuls, so their DMAs go at the
            # head of every queue
            xt_first = xpool.tile([128, DC, TT], f16, tag="xt",
                                  name="xt_first")
            for dc in range(DC):
                nc.sync.dma_start(xt_first[:, dc, :], xT_r[0, 0, dc])
                nc.sync.dma_start(wq_sb[:, dc, :], wq_r[dc])
                nc.sync.dma_start(wk_sb[:, dc, :], wk_r[dc])
            for dc in range(DC):
                # v-projection starts ~14us in; keep wv out of the q/k
                # projections' DMA window
                nc.sync.dma_start(wv_sb[:, dc, :], wv_r[dc])

            def load_consts():
                # emitted after the first x tile's DMAs: nothing here is
                # needed before RoPE / attention of the first tile
                for i in range(NTT):
                    sl = slice(i * TT, (i + 1) * TT)
                    nc.sync.dma_start(cos_sb[:, sl], cos.ap()[i])
                    nc.sync.dma_start(sin_sb[:, sl], sin.ap()[i])
                nc.sync.dma_start(tri_sb[:], tri.ap())
                nc.sync.dma_start(idn_sb[:], idn.ap())
                nc.sync.dma_start(onc_sb[:], onc.ap())
                nc.sync.dma_start(onr_sb[:], onr.ap())

            def load_wo():
                # not needed until the first out-projection fillers, so
                # keep these 2MB out of the first x-tile's DMA window
                for hh in range(HPC):
                    for nch in range(NCH):
                        nsl = slice(nch * 512, (nch + 1) * 512)
                        nc.sync.dma_start(wo_sb[:, hh, nsl], wo_r[hh, nch])

            # deferred PE work units; popped between attention tiles and
            # projection groups to keep the in-order PE queue saturated
            fillers = collections.deque()

            def pop_filler():
                if fillers:
                    fillers.popleft()()

            def make_yunit(b, qi, tc2, nch, otn_pair, cp_eng):
                def yunit():
                    yp = psY.tile([TK, 512], f32, tag="y", name="yp")
                    for hh in range(HPC):
                        nc.tensor.matmul(
                            yp[:],
                            otn_pair[hh][:, tc2 * TK:(tc2 + 1) * TK],
                            wo_sb[:, hh, nch * 512:(nch + 1) * 512],
                            start=(hh == 0), stop=(hh == HPC - 1),
                        )
                    ysb = ysbp.tile([TK, 512], f16, tag="ysb", name="ysb")
                    if cp_eng == 0:
                        nc.scalar.copy(ysb[:], yp[:])
                    else:
                        nc.vector.tensor_copy(ysb[:], yp[:])
                    tq0 = qi * TQ + tc2 * TK
                    nc.sync.dma_start(
                        y_r[b, tq0:tq0 + TK, nch * 512:(nch + 1) * 512],
                        ysb[:])
                return yunit

            xt_next = None  # prefetched first x tile of the next batch
            otn_tiles = {}

            for b in range(b_sz):
                # ---------------- phase A: projections + RoPE ----------
                qT = [qkv.tile([DH, t_sz], f16, tag=f"qT{h}", name=f"qT{h}")
                      for h in range(HPC)]
                kT = [qkv.tile([DH, t_sz], f16, tag=f"kT{h}", name=f"kT{h}")
                      for h in range(HPC)]
                vv = qkv.tile([128, NKT, HPC * DH], f16, tag="vv", name="vv")

                for tt in range(NTT):
                    tsl = slice(tt * TT, (tt + 1) * TT)
                    if tt == 0:
                        if b == 0:
                            xt = xt_first
                            load_consts()
                        else:
                            xt = xt_next
                            xt_next = None
                    else:
                        xt = xt_pf
                    if tt + 1 < NTT:
                        # prefetch the next x tile now so its DMA overlaps
                        # this tile's ~21us of projection matmuls
                        xt_pf = xpool.tile([128, DC, TT], f16, tag="xt",
                                           name="xt_pf2")
                        for dc in range(DC):
                            nc.sync.dma_start(xt_pf[:, dc, :],
                                              xT_r[b, tt + 1, dc])

                    if b == 0 and tt == 1:
                        # after xt(tt=1)'s own DMAs: wo is not needed until
                        # the first out-projection fillers in phase B
                        load_wo()

                    for h in range(HPC):
                        hs = slice(h * DH, (h + 1) * DH)
                        for dst, w_sb in ((qT[h], wq_sb), (kT[h], wk_sb)):
                            pp = psS.tile([TK, TQ], f32, tag="st", name="pp")
                            for dc in range(DC):
                                nc.tensor.matmul(
                                    pp[0:DH, :],
                                    w_sb[:, dc, hs],
                                    xt[:, dc, :],
                                    start=(dc == 0), stop=(dc == DC - 1),
                                )
                            # RoPE: dst = pp*cos + swap(pp)*sin_signed.
                            # The rotate-half swap needs mismatched base
                            # partitions, which walrus only allows when one
                            # operand is PSUM — so all three muls read pp
                            # from PSUM on DVE; the final all-SBUF fp16 add
                            # runs on the otherwise-idle GpSimd engine.
                            sh = rope.tile([DH, TT], f16, tag="sh", name="sh")
                            nc.vector.tensor_mul(
                                sh[0:64, :], pp[64:128, :],
                                sin_sb[0:64, tsl])
                            nc.vector.tensor_mul(
                                sh[64:128, :], pp[0:64, :],
                                sin_sb[64:128, tsl])
                            t1 = rope.tile([DH, TT], f16, tag="t1", name="t1")
                            nc.vector.scalar_tensor_tensor(
                                t1[:], pp[0:DH, :], 1.0, cos_sb[:, tsl],
                                BYP, MUL)
                            nc.vector.tensor_add(dst[:, tsl], t1[:], sh[:])
                            if len(fillers) > 12:
                                pop_filler()

                    for ts2 in range(TT // TK):
                        vp = psS.tile([TK, TQ], f32, tag="st", name="vp")
                        for dc in range(DC):
                            nc.tensor.matmul(
                                vp[:, 0:HPC * DH],
                                xt[:, dc, ts2 * TK:(ts2 + 1) * TK],
                                wv_sb[:, dc, :],
                                start=(dc == 0), stop=(dc == DC - 1),
                            )
                        kv_i = tt * (TT // TK) + ts2
                        nc.scalar.copy(vv[:, kv_i, :], vp[:, 0:HPC * DH])
                        if len(fillers) > 12:
                            pop_filler()

                # prefetch the first x tile of the next batch; by phase B
                # the input DMA queues are otherwise idle
                if b + 1 < b_sz:
                    xt_next = xpool.tile([128, DC, TT], f16, tag="xt",
                                         name="xt_pf")
                    for dc in range(DC):
                        nc.sync.dma_start(xt_next[:, dc, :],
                                          xT_r[b + 1, 0, dc])

                # ---------------- phase B: attention ------------------
                cp_rr = 0
                for h in range(HPC):
                    hs = slice(h * DH, (h + 1) * DH)
                    for qi in range(NQG):
                        nkv = KPG * (qi + 1)
                        q0 = qi * TQ
                        outp = psO.tile([DH, TQ], f32, tag="outT",
                                        name="outp")
                        sump = psR.tile([1, TQ], f32, tag="sums",
                                        name="sump")
                        sum_started = [False]
                        pend = [None]   # full-width pex awaiting its pair
                        pend2 = [None]  # pair buffer awaiting its quad
                        sum_q = []     # deferred sump matmuls

                        def sum_mm(src_ap, slo, last=False):
                            nc.tensor.matmul(
                                sump[0:1, slo:TQ], onc_sb[:],
                                src_ap[:, slo:TQ],
                                start=(not sum_started[0]), stop=last)
                            sum_started[0] = True
                        prevs = []
                        for ki in range(nkv):
                            dg = ki - KPG * qi
                            lo = max(dg, 0) * TK
                            stp = psS.tile([TK, TQ], f32, tag="st",
                                           name="stp")
                            nc.tensor.matmul(
                                stp[:, lo:TQ],
                                kT[h][:, ki * TK:(ki + 1) * TK],
                                qT[h][:, q0 + lo:q0 + TQ],
                                start=True, stop=(dg < 0),
                            )
                            if dg >= 0:
                                # additive triangle on the 128-wide strip
                                nc.tensor.matmul(
                                    stp[:, lo:lo + TK],
                                    idn_sb[:],
                                    tri_sb[:],
                                    start=False, stop=True,
                                )
                            pex = pexp.tile([TK, TQ], f16, tag="pex",
                                            name="pex")
                            nc.scalar.activation(pex[:, lo:TQ],
                                                 stp[:, lo:TQ], EXP)
                            # softmax denominators: full tiles are
                            # pair-summed on DVE (halves the add count),
                            # each pair / diagonal tile then accumulates
                            # into sump via a ones-column matmul. The
                            # matmul is deferred one tile so the PE never
                            # waits on the DVE pair-add (p-state guard).
                            if dg >= 0:
                                sum_q.append((pex, lo))
                            elif pend[0] is None:
                                pend[0] = pex
                            else:
                                pairb = accp.tile([TK, TQ], f16, tag="pair",
                                                  name="pairb")
                                nc.vector.scalar_tensor_tensor(
                                    pairb[:], pend[0][:], 1.0, pex[:],
                                    BYP, ADD)
                                pend[0] = None
                                if pend2[0] is None:
                                    pend2[0] = pairb
                                else:
                                    # quad: fold the two pair buffers so a
                                    # single ones-matmul covers 4 kv tiles
                                    nc.vector.scalar_tensor_tensor(
                                        pairb[:], pend2[0][:], 1.0,
                                        pairb[:], BYP, ADD)
                                    pend2[0] = None
                                    sum_q.append((pairb, 0))
                            if len(prevs) >= 2:
                                pk, plo, ppex = prevs.pop(0)
                                nc.tensor.matmul(
                                    outp[:, plo:TQ],
                                    vv[:, pk, hs],
                                    ppex[:, plo:TQ],
                                    start=(pk == 0), stop=False,
                                )
                            if len(sum_q) > 1:
                                s_ap, s_lo = sum_q.pop(0)
                                sum_mm(s_ap, s_lo)
                            pop_filler()
                            prevs.append((ki, lo, pex))
                        for di, (pk, plo, ppex) in enumerate(prevs):
                            nc.tensor.matmul(
                                outp[:, plo:TQ],
                                vv[:, pk, hs],
                                ppex[:, plo:TQ],
                                start=(pk == 0),
                                stop=(di == len(prevs) - 1),
                            )
                        while sum_q:
                            s_ap, s_lo = sum_q.pop(0)
                            sum_mm(s_ap, s_lo, last=(not sum_q))
                        oraw = sax.tile([DH, TQ], f16, tag="oraw",
                                        name="oraw")
                        nc.scalar.copy(oraw[:], outp[:])

                        def norm_filler(h=h, qi=qi, sump=sump,
                                        oraw=oraw, b=b):
                            # 1/denom as exp(-ln(denom)): same ACT table as
                            # Copy/Exp (no reloads), and deferred to this
                            # filler so the next group's exp0 is already
                            # ahead of it in the ACT queue
                            lnv = sax.tile([1, TQ], f32, tag="lnv",
                                           name="lnv")
                            nc.scalar.activation(lnv[0:1, :], sump[0:1, :],
                                                 LN)
                            rcp16 = sax.tile([1, TQ], f16, tag="rcp16",
                                             name="rcp16")
                            nc.scalar.activation(rcp16[0:1, :], lnv[0:1, :],
                                                 EXP, scale=-1.0)
                            rbc = psR.tile([DH, TQ], f32, tag="sums",
                                           name="rbc")
                            nc.tensor.matmul(rbc[:], onr_sb[:],
                                             rcp16[0:1, :],
                                             start=True, stop=True)
                            otn = otnp.tile([DH, TQ], f16, tag="otn",
                                            name="otn")
                            nc.vector.scalar_tensor_tensor(
                                otn[:], oraw[:], 1.0, rbc[:], BYP, MUL)
                            otn_tiles[(h, qi)] = otn
                            if h == HPC - 1:
                                nonlocal cp_rr
                                pair = (otn_tiles[(0, qi)], otn)
                                for tc2 in range(KPG):
                                    for nch in range(NCH):
                                        fillers.append(make_yunit(
                                            b, qi, tc2, nch, pair,
                                            (0 if cp_rr % 4 == 0 else 1)))
                                        cp_rr += 1

                        norm_filler.kind = "n"
                        # norm fillers go to the front (cheap, and they
                        # unblock the sump/rbc PSUM rotation) but in push
                        # order: h1's filler reads h0's otn
                        ni = 0
                        while (ni < len(fillers)
                               and getattr(fillers[ni], "kind", "y") == "n"):
                            ni += 1
                        fillers.insert(ni, norm_filler)
            # drain remaining deferred work
            while fillers:
                pop_filler()
    if legalize:
        _legalize_waits(nc, mybir)
    return nc


_NC_CACHE = {}
LAST_RESULT = None


def _get_nc(b_sz, t_sz, d_sz):
    key = (b_sz, t_sz, d_sz)
    if key not in _NC_CACHE:
        _NC_CACHE[key] = _build_nc(b_sz, t_sz, d_sz)
    return _NC_CACHE[key]


def kernel(x, w_q, w_k, w_v, w_o):
    from concourse.bass_utils import run_bass_kernel_spmd

    b_sz, t_sz, d_sz = x.shape
    scale = np.float32(1.0 / np.sqrt(DH))

    ntt, dc_n = t_sz // TT, d_sz // 128
    # pack to [B, NTT, DC, 128, TT]: each (tt, dc) chunk is one contiguous
    # 128KB DRAM run for the DMA engines
    xT = np.asarray(x, np.float32).astype(np.float16)
    xT = xT.transpose(0, 2, 1).reshape(b_sz, dc_n, 128, ntt, TT)
    xT = np.ascontiguousarray(xT.transpose(0, 3, 1, 2, 4))
    w_q = np.asarray(w_q, np.float32)
    w_k = np.asarray(w_k, np.float32)
    w_v = np.asarray(w_v, np.float32)
    w_o = np.asarray(w_o, np.float32)
    cosT, sinT = _rope_tables(t_sz, DH, THETA)
    cosP = np.ascontiguousarray(
        cosT.reshape(DH, ntt, TT).transpose(1, 0, 2))
    sinP = np.ascontiguousarray(
        sinT.reshape(DH, ntt, TT).transpose(1, 0, 2))

    def pack_w(w):  # [D, 256] -> [DC, 128, 256]
        return np.ascontiguousarray(
            w.astype(np.float16).reshape(dc_n, 128, HPC * DH))

    def pack_wo(w):  # [256, D] -> [HPC, NCH, 128, 512]
        w = w.astype(np.float16).reshape(HPC, 128, d_sz // 512, 512)
        return np.ascontiguousarray(w.transpose(0, 2, 1, 3))
    trim = np.zeros((TK, TK), np.float16)
    for r in range(TK):
        trim[r, :r] = MASKV
    ident = np.eye(128, dtype=np.float16)

    in_maps = []
    for c in range(NCORES):
        cs = slice(c * HPC * DH, (c + 1) * HPC * DH)
        in_maps.append({
            "xT": xT,
            "wq": pack_w(w_q[:, cs] * scale),
            "wk": pack_w(w_k[:, cs]),
            "wv": pack_w(w_v[:, cs]),
            "wo": pack_wo(w_o[cs, :]),
            "cos": cosP,
            "sin": sinP,
            "tri": trim,
            "idn": ident,
            "onc": np.ones((128, 1), np.float16),
            "onr": np.ones((1, 128), np.float16),
        })

    nc = _get_nc(b_sz, t_sz, d_sz)
    res = run_bass_kernel_spmd(nc, in_maps, core_ids=list(range(NCORES)))
    global LAST_RESULT
    LAST_RESULT = res

    out = res.results[0]["y"].astype(np.float32)
    for c in range(1, NCORES):
        out += res.results[c]["y"].astype(np.float32)
    return out
